# revision 7
# baseline (speedup 1.0000x reference)
"""BlockSparseLinear on 8 TRN2 NeuronCores.

Computes out = x @ W_dense.T + bias where W_dense is a [4096, 4096] matrix
assembled from 8192 nonzero 32x32 blocks (50% density).

Strategy:
  - Host: scatter the nonzero blocks into dense per-core weight shards, in the
    exact transposed/tiled DRAM layout the device kernel wants.
  - Sharding: 4-way over tokens x 2-way over out-features (8 cores).
    Per core: out_shard[1024 tokens, 2048 outf] = x_shard @ W_half.T + bias.
  - Device: dense matmul in float32r (FP22 reduced-precision fp32, full PE
    rate at moving-dim >= 256). x^T shard stays SBUF-resident (16 MB);
    weight tiles stream from HBM.

Per-core loop nest (natural orientation, out[tokens, outf]):
  for n in 4 (512-wide outf chunks):
    psum[m] for m in 8 token-tiles of 128   (8 PSUM banks)
    for k in 32 (128-wide contraction tiles):
      w_sb = DMA W_T[n, k]                  ([128 k, 512 o])
      for m in 8: matmul(psum[m], lhsT=xT[k, m], rhs=w_sb, start=k==0, stop=k==31)
    for m in 8: out_sb = psum[m] + bias[n-chunk]; DMA to HBM
"""

import os

import numpy as np

import concourse.mybir as mybir
import concourse.tile as tile
from concourse import bacc
from concourse.bass_utils import run_bass_kernel_spmd

BLOCK = 32
IN_FEATURES = 4096
OUT_FEATURES = 4096
N_TOKENS = 4096
IN_BLOCKS = IN_FEATURES // BLOCK  # 128
OUT_BLOCKS = OUT_FEATURES // BLOCK  # 128

N_CORES = 8
T_SHARDS = 4  # token shards
O_SHARDS = 2  # out-feature shards
TSH = N_TOKENS // T_SHARDS  # 1024 tokens per core
OSH = OUT_FEATURES // O_SHARDS  # 2048 out features per core

P = 128  # partitions
NFREE = 512  # matmul moving free dim (one PSUM bank of fp32)
K_TILES = IN_FEATURES // P  # 32
M_TILES = TSH // P  # 8 token tiles per core
N_CHUNKS = OSH // NFREE  # 4 outf chunks per core

# exec time of the slowest core from the last traced run (ns), None if untraced
LAST_EXEC_NS = None
LAST_RESULT = None


def _install_axon_ntff_hook():
    """Best-effort: register the axon NTFF profiling hook that the image's
    antenv package lacks. Returns True if tracing is possible."""
    try:
        from antenv.axon_hooks import get_axon_ntff_profile_hook

        return get_axon_ntff_profile_hook() is not None
    except ImportError:
        pass
    try:
        import sys
        import types

        import antenv
        import trn_agent_boot.trn_boot as tb

        hook = tb._ntff_profile_via_ctypes("/opt/axon/libaxon_pjrt.so")
        if hook is None:
            return False
        mod = types.ModuleType("antenv.axon_hooks")
        mod._hook = hook
        mod.get_axon_ntff_profile_hook = lambda: mod._hook
        mod.set_axon_ntff_profile_hook = lambda h: setattr(mod, "_hook", h)
        sys.modules["antenv.axon_hooks"] = mod
        antenv.axon_hooks = mod

        # avoid the artifact-upload dependency in the trace path
        import concourse.bass_utils as bu

        bu.upload_artifacts = lambda tmpdir: str(tmpdir)
        return True
    except Exception:
        return False


def _build_bass():
    nc = bacc.Bacc(None, target_bir_lowering=False)

    x_d = nc.dram_tensor(
        "xt", [P, K_TILES, TSH], mybir.dt.float32r, kind="ExternalInput"
    )
    w_d = nc.dram_tensor(
        "wt", [N_CHUNKS, K_TILES, P, NFREE], mybir.dt.float32r, kind="ExternalInput"
    )
    b_d = nc.dram_tensor("bias", [P, OSH], mybir.dt.float32, kind="ExternalInput")
    o_d = nc.dram_tensor(
        "out", [M_TILES, P, N_CHUNKS, NFREE], mybir.dt.float32, kind="ExternalOutput"
    )

    with tile.TileContext(nc) as tc:
        with (
            tc.tile_pool(name="xpool", bufs=1) as xpool,
            tc.tile_pool(name="wpool", bufs=4) as wpool,
            tc.tile_pool(name="opool", bufs=4) as opool,
            tc.tile_pool(name="bpool", bufs=1) as bpool,
            tc.tile_pool(name="psum", bufs=8, space="PSUM") as ppool,
        ):
            bias_sb = bpool.tile([P, OSH], mybir.dt.float32)
            nc.sync.dma_start(bias_sb[:], b_d[:])

            x_sb = xpool.tile([P, K_TILES, TSH], mybir.dt.float32r)
            for k in range(K_TILES):
                nc.sync.dma_start(x_sb[:, k], x_d[:, k])

            for n in range(N_CHUNKS):
                psums = [
                    ppool.tile([P, NFREE], mybir.dt.float32, tag="acc", name="acc")
                    for _ in range(M_TILES)
                ]
                for k in range(K_TILES):
                    w_sb = wpool.tile([P, NFREE], mybir.dt.float32r, tag="w", name="w")
                    nc.sync.dma_start(w_sb[:], w_d[n, k])
                    for m in range(M_TILES):
                        nc.tensor.matmul(
                            psums[m][:],
                            lhsT=x_sb[:, k, m * P : (m + 1) * P],
                            rhs=w_sb[:],
                            start=(k == 0),
                            stop=(k == K_TILES - 1),
                        )
                for m in range(M_TILES):
                    o_sb = opool.tile([P, NFREE], mybir.dt.float32, tag="o", name="o")
                    nc.vector.tensor_add(
                        out=o_sb[:],
                        in0=psums[m][:],
                        in1=bias_sb[:, n * NFREE : (n + 1) * NFREE],
                    )
                    nc.sync.dma_start(o_d[m, :, n], o_sb[:])

    nc.compile()
    return nc


def _dense_weight(weight_data, block_ids):
    """Scatter nonzero 32x32 blocks into dense [OUT, IN] (numpy, host-side)."""
    w = np.zeros((OUT_FEATURES, IN_FEATURES), dtype=np.float32)
    br = block_ids.astype(np.int64) // IN_BLOCKS
    bc = block_ids.astype(np.int64) % IN_BLOCKS
    # view as [OUT_BLOCKS, 32, IN_BLOCKS, 32] and scatter per-block
    w4 = w.reshape(OUT_BLOCKS, BLOCK, IN_BLOCKS, BLOCK)
    w4[br, :, bc, :] = weight_data
    return w


def kernel(x, weight_data, bias, block_ids):
    x = np.ascontiguousarray(np.asarray(x, dtype=np.float32))
    weight_data = np.asarray(weight_data, dtype=np.float32)
    bias = np.asarray(bias, dtype=np.float32)
    block_ids = np.asarray(block_ids)

    w = _dense_weight(weight_data, block_ids)  # [OUT, IN]

    # per-token-shard x^T in device layout [P, K_TILES, TSH]:
    # xt[p, k, t] = x[t0 + t, k*128 + p]
    xts = []
    for ti in range(T_SHARDS):
        xs = x[ti * TSH : (ti + 1) * TSH, :]  # [TSH, IN]
        xt = np.ascontiguousarray(
            xs.T.reshape(K_TILES, P, TSH).transpose(1, 0, 2)
        )  # [P, K_TILES, TSH]
        xts.append(xt)

    # per-outf-shard W^T in device layout [N_CHUNKS, K_TILES, P, NFREE]:
    # wt[n, k, p, o] = W[o0 + n*512 + o, k*128 + p]
    wts = []
    biases = []
    for oi in range(O_SHARDS):
        ws = w[oi * OSH : (oi + 1) * OSH, :]  # [OSH, IN]
        wt = np.ascontiguousarray(
            ws.reshape(N_CHUNKS, NFREE, K_TILES, P).transpose(0, 2, 3, 1)
        )
        wts.append(wt)
        bs = bias[oi * OSH : (oi + 1) * OSH]
        biases.append(np.ascontiguousarray(np.broadcast_to(bs[None, :], (P, OSH))))

    in_maps = []
    for c in range(N_CORES):
        ti, oi = c // O_SHARDS, c % O_SHARDS
        in_maps.append({"xt": xts[ti], "wt": wts[oi], "bias": biases[oi]})

    nc = _build_bass()
    trace = bool(int(os.environ.get("BSL_TRACE", "0")))
    if trace:
        trace = _install_axon_ntff_hook()
    kwargs = {}
    if trace:
        tdir = os.environ.get("BSL_TRACE_DIR")
        if tdir:
            os.makedirs(tdir, exist_ok=True)
            kwargs["tmpdir"] = tdir
        kwargs["trace_cores"] = list(range(N_CORES))
    res = run_bass_kernel_spmd(
        nc,
        in_maps,
        core_ids=list(range(N_CORES)),
        trace=trace,
        **kwargs,
    )

    global LAST_EXEC_NS, LAST_RESULT
    LAST_EXEC_NS = res.exec_time_ns
    LAST_RESULT = res

    out = np.empty((N_TOKENS, OUT_FEATURES), dtype=np.float32)
    for c in range(N_CORES):
        ti, oi = c // O_SHARDS, c % O_SHARDS
        o = res.results[c]["out"]  # [M_TILES, P, N_CHUNKS, NFREE]
        o = o.transpose(0, 1, 2, 3).reshape(M_TILES * P, N_CHUNKS * NFREE)
        out[ti * TSH : (ti + 1) * TSH, oi * OSH : (oi + 1) * OSH] = o
    return out


# revision 8
# speedup vs baseline: 1.0276x; 1.0276x over previous
"""BlockSparseLinear on 8 TRN2 NeuronCores.

Computes out = x @ W_dense.T + bias where W_dense is a [4096, 4096] matrix
assembled from 8192 nonzero 32x32 blocks (50% density).

Strategy:
  - Host: scatter the nonzero blocks into dense per-core weight shards, in the
    exact transposed/tiled DRAM layout the device kernel wants.
  - Sharding: 4-way over tokens x 2-way over out-features (8 cores).
    Per core: out_shard[1024 tokens, 2048 outf] = x_shard @ W_half.T + bias.
  - Device: dense matmul in float32r (FP22 reduced-precision fp32, full PE
    rate at moving-dim >= 256). x^T shard is SBUF-resident as 32 per-k tiles
    (so compute starts as soon as the first k-slice lands); weight tiles
    stream from HBM batched 4 o-tiles per DMA.

Per-core loop nest (out^T orientation: psum[o-partitions, token-free]):
  for ob in 4 (groups of 4 o-tiles of 128 outf):
    psum[oi][tc] for oi in 4, tc in 2      (8 PSUM banks)
    for k in 32 (128-wide contraction tiles):
      w_sb = DMA W[ob, k]                  ([128 k, 4 oi, 128 o], one 256KB DMA)
      for oi in 4, tc in 2:
        matmul(psum[oi][tc], lhsT=w_sb[:, oi], rhs=x_k[tc*512:], start/stop)
    for oi, tc: out_sb = psum + bias[o-tile] (DVE); DMA out^T (ACT ring)
"""

import os

import numpy as np

import concourse.mybir as mybir
import concourse.tile as tile
from concourse import bacc
from concourse.bass_utils import run_bass_kernel_spmd

BLOCK = 32
IN_FEATURES = 4096
OUT_FEATURES = 4096
N_TOKENS = 4096
IN_BLOCKS = IN_FEATURES // BLOCK  # 128
OUT_BLOCKS = OUT_FEATURES // BLOCK  # 128

N_CORES = 8
T_SHARDS = 4  # token shards
O_SHARDS = 2  # out-feature shards
TSH = N_TOKENS // T_SHARDS  # 1024 tokens per core
OSH = OUT_FEATURES // O_SHARDS  # 2048 out features per core

P = 128  # partitions
NFREE = 512  # matmul moving free dim (one PSUM bank of fp32)
K_TILES = IN_FEATURES // P  # 32
T_CHUNKS = TSH // NFREE  # 2 moving token chunks per core
O_TILES = OSH // P  # 16 o-tiles of 128 outf
OB_GROUPS = 4  # o-tile groups (phases)
OB_SIZE = O_TILES // OB_GROUPS  # 4 o-tiles per phase

# exec time of the slowest core from the last traced run (ns), None if untraced
LAST_EXEC_NS = None
LAST_RESULT = None


def _install_axon_ntff_hook():
    """Best-effort: register the axon NTFF profiling hook that the image's
    antenv package lacks. Returns True if tracing is possible."""
    try:
        from antenv.axon_hooks import get_axon_ntff_profile_hook

        return get_axon_ntff_profile_hook() is not None
    except ImportError:
        pass
    try:
        import sys
        import types

        import antenv
        import trn_agent_boot.trn_boot as tb

        hook = tb._ntff_profile_via_ctypes("/opt/axon/libaxon_pjrt.so")
        if hook is None:
            return False
        mod = types.ModuleType("antenv.axon_hooks")
        mod._hook = hook
        mod.get_axon_ntff_profile_hook = lambda: mod._hook
        mod.set_axon_ntff_profile_hook = lambda h: setattr(mod, "_hook", h)
        sys.modules["antenv.axon_hooks"] = mod
        antenv.axon_hooks = mod

        # avoid the artifact-upload dependency in the trace path
        import concourse.bass_utils as bu

        bu.upload_artifacts = lambda tmpdir: str(tmpdir)
        return True
    except Exception:
        return False


def _build_bass():
    nc = bacc.Bacc(None, target_bir_lowering=False)

    x_d = nc.dram_tensor(
        "xt", [P, K_TILES, TSH], mybir.dt.float32r, kind="ExternalInput"
    )
    w_d = nc.dram_tensor(
        "wt",
        [OB_GROUPS, K_TILES, P, OB_SIZE, P],
        mybir.dt.float32r,
        kind="ExternalInput",
    )
    b_d = nc.dram_tensor("bias", [P, O_TILES], mybir.dt.float32, kind="ExternalInput")
    o_d = nc.dram_tensor(
        "out", [O_TILES, T_CHUNKS, P, NFREE], mybir.dt.float32, kind="ExternalOutput"
    )

    with tile.TileContext(nc) as tc:
        with (
            tc.tile_pool(name="xpool", bufs=1) as xpool,
            tc.tile_pool(name="wpool", bufs=4) as wpool,
            tc.tile_pool(name="opool", bufs=4) as opool,
            tc.tile_pool(name="bpool", bufs=1) as bpool,
            tc.tile_pool(name="psum", bufs=8, space="PSUM") as ppool,
        ):
            bias_sb = bpool.tile([P, O_TILES], mybir.dt.float32)
            nc.sync.dma_start(bias_sb[:], b_d[:])

            # x resident as 32 per-k tiles so deps are per-slice
            x_tiles = []
            for k in range(K_TILES):
                x_k = xpool.tile([P, TSH], mybir.dt.float32r, tag=f"x{k}", name="x")
                nc.sync.dma_start(x_k[:], x_d[:, k])
                x_tiles.append(x_k)

            for ob in range(OB_GROUPS):
                psums = [
                    [
                        ppool.tile([P, NFREE], mybir.dt.float32, tag="acc", name="acc")
                        for _ in range(T_CHUNKS)
                    ]
                    for _ in range(OB_SIZE)
                ]
                for k in range(K_TILES):
                    w_sb = wpool.tile(
                        [P, OB_SIZE, P], mybir.dt.float32r, tag="w", name="w"
                    )
                    nc.sync.dma_start(w_sb[:], w_d[ob, k])
                    for oi in range(OB_SIZE):
                        for tcn in range(T_CHUNKS):
                            nc.tensor.matmul(
                                psums[oi][tcn][:],
                                lhsT=w_sb[:, oi],
                                rhs=x_tiles[k][:, tcn * NFREE : (tcn + 1) * NFREE],
                                start=(k == 0),
                                stop=(k == K_TILES - 1),
                            )
                for oi in range(OB_SIZE):
                    ot = ob * OB_SIZE + oi
                    for tcn in range(T_CHUNKS):
                        o_sb = opool.tile([P, NFREE], mybir.dt.float32, tag="o", name="o")
                        nc.vector.tensor_tensor(
                            o_sb[:],
                            psums[oi][tcn][:],
                            bias_sb[:, ot : ot + 1].to_broadcast([P, NFREE]),
                            mybir.AluOpType.add,
                        )
                        nc.scalar.dma_start(o_d[ot, tcn], o_sb[:])

    nc.compile()
    return nc


def _dense_weight(weight_data, block_ids):
    """Scatter nonzero 32x32 blocks into dense [OUT, IN] (numpy, host-side)."""
    w = np.zeros((OUT_FEATURES, IN_FEATURES), dtype=np.float32)
    br = block_ids.astype(np.int64) // IN_BLOCKS
    bc = block_ids.astype(np.int64) % IN_BLOCKS
    # view as [OUT_BLOCKS, 32, IN_BLOCKS, 32] and scatter per-block
    w4 = w.reshape(OUT_BLOCKS, BLOCK, IN_BLOCKS, BLOCK)
    w4[br, :, bc, :] = weight_data
    return w


def kernel(x, weight_data, bias, block_ids):
    x = np.ascontiguousarray(np.asarray(x, dtype=np.float32))
    weight_data = np.asarray(weight_data, dtype=np.float32)
    bias = np.asarray(bias, dtype=np.float32)
    block_ids = np.asarray(block_ids)

    w = _dense_weight(weight_data, block_ids)  # [OUT, IN]

    # per-token-shard x^T in device layout [P, K_TILES, TSH]:
    # xt[p, k, t] = x[t0 + t, k*128 + p]
    xts = []
    for ti in range(T_SHARDS):
        xs = x[ti * TSH : (ti + 1) * TSH, :]  # [TSH, IN]
        xt = np.ascontiguousarray(
            xs.T.reshape(K_TILES, P, TSH).transpose(1, 0, 2)
        )  # [P, K_TILES, TSH]
        xts.append(xt)

    # per-outf-shard W in device layout [OB_GROUPS, K_TILES, P(k), OB_SIZE, P(o)]:
    # wt[ob, k, p, oi, o] = W[o0 + (ob*OB_SIZE + oi)*128 + o, k*128 + p]
    wts = []
    biases = []
    for si in range(O_SHARDS):
        ws = w[si * OSH : (si + 1) * OSH, :]  # [OSH, IN]
        # [ot, o, k, p] -> [ob, oi, o, k, p] -> [ob, k, p, oi, o]
        wt = ws.reshape(O_TILES, P, K_TILES, P).reshape(
            OB_GROUPS, OB_SIZE, P, K_TILES, P
        )
        wt = np.ascontiguousarray(wt.transpose(0, 3, 4, 1, 2))
        wts.append(wt)
        bs = bias[si * OSH : (si + 1) * OSH]  # [OSH]
        biases.append(np.ascontiguousarray(bs.reshape(O_TILES, P).T))  # [P, O_TILES]

    in_maps = []
    for c in range(N_CORES):
        ti, si = c // O_SHARDS, c % O_SHARDS
        in_maps.append({"xt": xts[ti], "wt": wts[si], "bias": biases[si]})

    nc = _build_bass()
    trace = bool(int(os.environ.get("BSL_TRACE", "0")))
    if trace:
        trace = _install_axon_ntff_hook()
    kwargs = {}
    if trace:
        tdir = os.environ.get("BSL_TRACE_DIR")
        if tdir:
            os.makedirs(tdir, exist_ok=True)
            kwargs["tmpdir"] = tdir
        kwargs["trace_cores"] = list(range(N_CORES))
    res = run_bass_kernel_spmd(
        nc,
        in_maps,
        core_ids=list(range(N_CORES)),
        trace=trace,
        **kwargs,
    )

    global LAST_EXEC_NS, LAST_RESULT
    LAST_EXEC_NS = res.exec_time_ns
    LAST_RESULT = res

    out = np.empty((N_TOKENS, OUT_FEATURES), dtype=np.float32)
    for c in range(N_CORES):
        ti, si = c // O_SHARDS, c % O_SHARDS
        o = res.results[c]["out"]  # [O_TILES, T_CHUNKS, P(o), NFREE(t)]
        # -> out_shard[t, o]: t = tc*512 + j, o = ot*128 + p
        o = o.transpose(1, 3, 0, 2).reshape(TSH, OSH)
        out[ti * TSH : (ti + 1) * TSH, si * OSH : (si + 1) * OSH] = o
    return out


# revision 9
# speedup vs baseline: 1.1706x; 1.1392x over previous
"""BlockSparseLinear on 8 TRN2 NeuronCores.

Computes out = x @ W_dense.T + bias where W_dense is a [4096, 4096] matrix
assembled from 8192 nonzero 32x32 blocks (50% density).

Strategy:
  - Host: scatter the nonzero blocks into dense per-core weight shards, in the
    exact transposed/tiled DRAM layout the device kernel wants.
  - Sharding: 4-way over tokens x 2-way over out-features (8 cores).
    Per core: out_shard[1024 tokens, 2048 outf] = x_shard @ W_half.T + bias.
  - Device: dense matmul in float32r (FP22 reduced-precision fp32, full PE
    rate at moving-dim >= 256). x^T shard is SBUF-resident as 32 per-k tiles
    (so compute starts as soon as the first k-slice lands); weight tiles
    stream from HBM batched 4 o-tiles per DMA.

Per-core loop nest (out^T orientation: psum[o-partitions, token-free]):
  for ob in 4 (groups of 4 o-tiles of 128 outf):
    psum[oi][tc] for oi in 4, tc in 2      (8 PSUM banks)
    for k in 32 (128-wide contraction tiles):
      w_sb = DMA W[ob, k]                  ([128 k, 4 oi, 128 o], one 256KB DMA)
      for oi in 4, tc in 2:
        matmul(psum[oi][tc], lhsT=w_sb[:, oi], rhs=x_k[tc*512:], start/stop)
    for oi, tc: out_sb = psum + bias[o-tile] (DVE); DMA out^T (ACT ring)
"""

import os

import numpy as np

import concourse.mybir as mybir
import concourse.tile as tile
from concourse import bacc
from concourse.bass_utils import run_bass_kernel_spmd

BLOCK = 32
IN_FEATURES = 4096
OUT_FEATURES = 4096
N_TOKENS = 4096
IN_BLOCKS = IN_FEATURES // BLOCK  # 128
OUT_BLOCKS = OUT_FEATURES // BLOCK  # 128

N_CORES = 8
T_SHARDS = 4  # token shards
O_SHARDS = 2  # out-feature shards
TSH = N_TOKENS // T_SHARDS  # 1024 tokens per core
OSH = OUT_FEATURES // O_SHARDS  # 2048 out features per core

P = 128  # partitions
NFREE = 512  # matmul moving free dim (one PSUM bank of fp32)
K_TILES = IN_FEATURES // P  # 32
T_CHUNKS = TSH // NFREE  # 2 moving token chunks per core
O_TILES = OSH // P  # 16 o-tiles of 128 outf
OB_GROUPS = 4  # o-tile groups (phases)
OB_SIZE = O_TILES // OB_GROUPS  # 4 o-tiles per phase

# exec time of the slowest core from the last traced run (ns), None if untraced
LAST_EXEC_NS = None
LAST_RESULT = None


def _install_axon_ntff_hook():
    """Best-effort: register the axon NTFF profiling hook that the image's
    antenv package lacks. Returns True if tracing is possible."""
    try:
        from antenv.axon_hooks import get_axon_ntff_profile_hook

        return get_axon_ntff_profile_hook() is not None
    except ImportError:
        pass
    try:
        import sys
        import types

        import antenv
        import trn_agent_boot.trn_boot as tb

        hook = tb._ntff_profile_via_ctypes("/opt/axon/libaxon_pjrt.so")
        if hook is None:
            return False
        mod = types.ModuleType("antenv.axon_hooks")
        mod._hook = hook
        mod.get_axon_ntff_profile_hook = lambda: mod._hook
        mod.set_axon_ntff_profile_hook = lambda h: setattr(mod, "_hook", h)
        sys.modules["antenv.axon_hooks"] = mod
        antenv.axon_hooks = mod

        # avoid the artifact-upload dependency in the trace path
        import concourse.bass_utils as bu

        bu.upload_artifacts = lambda tmpdir: str(tmpdir)
        return True
    except Exception:
        return False


def _build_bass():
    nc = bacc.Bacc(None, target_bir_lowering=False)

    x_d = nc.dram_tensor(
        "xt", [P, K_TILES, TSH], mybir.dt.float32r, kind="ExternalInput"
    )
    w_d = nc.dram_tensor(
        "wt",
        [OB_GROUPS, K_TILES, P, OB_SIZE, P],
        mybir.dt.float32r,
        kind="ExternalInput",
    )
    b_d = nc.dram_tensor("bias", [P, O_TILES], mybir.dt.float32, kind="ExternalInput")
    o_d = nc.dram_tensor(
        "out", [O_TILES, T_CHUNKS, P, NFREE], mybir.dt.float32, kind="ExternalOutput"
    )

    with tile.TileContext(nc) as tc:
        with (
            tc.tile_pool(name="xpool", bufs=1) as xpool,
            tc.tile_pool(name="wpool", bufs=8) as wpool,
            tc.tile_pool(name="opool", bufs=4) as opool,
            tc.tile_pool(name="bpool", bufs=1) as bpool,
            tc.tile_pool(name="psum", bufs=8, space="PSUM") as ppool,
        ):
            bias_sb = bpool.tile([P, O_TILES], mybir.dt.float32)
            nc.scalar.dma_start(bias_sb[:], b_d[:])

            # x resident as 32 per-k tiles so deps are per-slice
            x_tiles = []
            for k in range(K_TILES):
                x_k = xpool.tile([P, TSH], mybir.dt.float32r, tag=f"x{k}", name="x")
                nc.scalar.dma_start(x_k[:], x_d[:, k])
                x_tiles.append(x_k)

            for ob in range(OB_GROUPS):
                psums = [
                    [
                        ppool.tile([P, NFREE], mybir.dt.float32, tag="acc", name="acc")
                        for _ in range(T_CHUNKS)
                    ]
                    for _ in range(OB_SIZE)
                ]
                for k in range(K_TILES):
                    w_sb = wpool.tile(
                        [P, OB_SIZE, P], mybir.dt.float32r, tag="w", name="w"
                    )
                    nc.sync.dma_start(w_sb[:], w_d[ob, k])
                    for oi in range(OB_SIZE):
                        for tcn in range(T_CHUNKS):
                            nc.tensor.matmul(
                                psums[oi][tcn][:],
                                lhsT=w_sb[:, oi],
                                rhs=x_tiles[k][:, tcn * NFREE : (tcn + 1) * NFREE],
                                start=(k == 0),
                                stop=(k == K_TILES - 1),
                            )
                for oi in range(OB_SIZE):
                    ot = ob * OB_SIZE + oi
                    for tcn in range(T_CHUNKS):
                        o_sb = opool.tile([P, NFREE], mybir.dt.float32, tag="o", name="o")
                        nc.vector.tensor_tensor(
                            o_sb[:],
                            psums[oi][tcn][:],
                            bias_sb[:, ot : ot + 1].to_broadcast([P, NFREE]),
                            mybir.AluOpType.add,
                        )
                        nc.gpsimd.dma_start(o_d[ot, tcn], o_sb[:])

    nc.compile()
    return nc


def _dense_weight(weight_data, block_ids):
    """Scatter nonzero 32x32 blocks into dense [OUT, IN] (numpy, host-side)."""
    w = np.zeros((OUT_FEATURES, IN_FEATURES), dtype=np.float32)
    br = block_ids.astype(np.int64) // IN_BLOCKS
    bc = block_ids.astype(np.int64) % IN_BLOCKS
    # view as [OUT_BLOCKS, 32, IN_BLOCKS, 32] and scatter per-block
    w4 = w.reshape(OUT_BLOCKS, BLOCK, IN_BLOCKS, BLOCK)
    w4[br, :, bc, :] = weight_data
    return w


def kernel(x, weight_data, bias, block_ids):
    x = np.ascontiguousarray(np.asarray(x, dtype=np.float32))
    weight_data = np.asarray(weight_data, dtype=np.float32)
    bias = np.asarray(bias, dtype=np.float32)
    block_ids = np.asarray(block_ids)

    w = _dense_weight(weight_data, block_ids)  # [OUT, IN]

    # per-token-shard x^T in device layout [P, K_TILES, TSH]:
    # xt[p, k, t] = x[t0 + t, k*128 + p]
    xts = []
    for ti in range(T_SHARDS):
        xs = x[ti * TSH : (ti + 1) * TSH, :]  # [TSH, IN]
        xt = np.ascontiguousarray(
            xs.T.reshape(K_TILES, P, TSH).transpose(1, 0, 2)
        )  # [P, K_TILES, TSH]
        xts.append(xt)

    # per-outf-shard W in device layout [OB_GROUPS, K_TILES, P(k), OB_SIZE, P(o)]:
    # wt[ob, k, p, oi, o] = W[o0 + (ob*OB_SIZE + oi)*128 + o, k*128 + p]
    wts = []
    biases = []
    for si in range(O_SHARDS):
        ws = w[si * OSH : (si + 1) * OSH, :]  # [OSH, IN]
        # [ot, o, k, p] -> [ob, oi, o, k, p] -> [ob, k, p, oi, o]
        wt = ws.reshape(O_TILES, P, K_TILES, P).reshape(
            OB_GROUPS, OB_SIZE, P, K_TILES, P
        )
        wt = np.ascontiguousarray(wt.transpose(0, 3, 4, 1, 2))
        wts.append(wt)
        bs = bias[si * OSH : (si + 1) * OSH]  # [OSH]
        biases.append(np.ascontiguousarray(bs.reshape(O_TILES, P).T))  # [P, O_TILES]

    in_maps = []
    for c in range(N_CORES):
        ti, si = c // O_SHARDS, c % O_SHARDS
        in_maps.append({"xt": xts[ti], "wt": wts[si], "bias": biases[si]})

    nc = _build_bass()
    trace = bool(int(os.environ.get("BSL_TRACE", "0")))
    if trace:
        trace = _install_axon_ntff_hook()
    kwargs = {}
    if trace:
        tdir = os.environ.get("BSL_TRACE_DIR")
        if tdir:
            os.makedirs(tdir, exist_ok=True)
            kwargs["tmpdir"] = tdir
        kwargs["trace_cores"] = list(range(N_CORES))
    res = run_bass_kernel_spmd(
        nc,
        in_maps,
        core_ids=list(range(N_CORES)),
        trace=trace,
        **kwargs,
    )

    global LAST_EXEC_NS, LAST_RESULT
    LAST_EXEC_NS = res.exec_time_ns
    LAST_RESULT = res

    out = np.empty((N_TOKENS, OUT_FEATURES), dtype=np.float32)
    for c in range(N_CORES):
        ti, si = c // O_SHARDS, c % O_SHARDS
        o = res.results[c]["out"]  # [O_TILES, T_CHUNKS, P(o), NFREE(t)]
        # -> out_shard[t, o]: t = tc*512 + j, o = ot*128 + p
        o = o.transpose(1, 3, 0, 2).reshape(TSH, OSH)
        out[ti * TSH : (ti + 1) * TSH, si * OSH : (si + 1) * OSH] = o
    return out


# revision 10
# speedup vs baseline: 1.2476x; 1.0658x over previous
"""BlockSparseLinear on 8 TRN2 NeuronCores.

Computes out = x @ W_dense.T + bias where W_dense is a [4096, 4096] matrix
assembled from 8192 nonzero 32x32 blocks (50% density).

Strategy:
  - Host: scatter the nonzero blocks into dense per-core weight shards, in the
    exact transposed/tiled DRAM layout the device kernel wants.
  - Sharding: 4-way over tokens x 2-way over out-features (8 cores).
    Per core: out_shard[1024 tokens, 2048 outf] = x_shard @ W_half.T + bias.
  - Device: dense matmul in float32r (FP22 reduced-precision fp32, full PE
    rate at moving-dim >= 256), out^T orientation (psum[o-part, token-free],
    stationary = weight tile, moving = x tile).
  - Phases iterate over k-blocks (8 contraction tiles each) sweeping ALL 16
    o-tiles, with per-o-tile fp32 SBUF accumulators. This spreads the x and W
    HBM traffic evenly across the whole kernel (~12MB per ~58us phase) instead
    of concentrating it in the first phase. Bias is folded into the first
    phase's PSUM->SBUF accumulate; x DMAs ride the ACT HWDGE ring, W DMAs the
    SP ring, so neither queues behind the other.

Per-core loop nest:
  for kb in 4 (k-blocks of 8 k-tiles):
    for ot in 16 (o-tiles of 128 outf):
      psum[tc] for tc in 2                  (2 PSUM banks, pipelined over ot)
      for k in kb, tc in 2:
        matmul(psum[tc], lhsT=w[kb,ot,k], rhs=x_k[tc*512:], start/stop at kb edges)
      acc[ot] (+)= psum (+ bias at kb==0)   (DVE)
      if kb == 3: DMA acc[ot] -> out^T      (ACT ring)
"""

import os

import numpy as np

import concourse.mybir as mybir
import concourse.tile as tile
from concourse import bacc
from concourse.bass_utils import run_bass_kernel_spmd

BLOCK = 32
IN_FEATURES = 4096
OUT_FEATURES = 4096
N_TOKENS = 4096
IN_BLOCKS = IN_FEATURES // BLOCK  # 128
OUT_BLOCKS = OUT_FEATURES // BLOCK  # 128

N_CORES = 8
T_SHARDS = 4  # token shards
O_SHARDS = 2  # out-feature shards
TSH = N_TOKENS // T_SHARDS  # 1024 tokens per core
OSH = OUT_FEATURES // O_SHARDS  # 2048 out features per core

P = 128  # partitions
NFREE = 512  # matmul moving free dim (one PSUM bank of fp32)
K_TILES = IN_FEATURES // P  # 32
T_CHUNKS = TSH // NFREE  # 2 moving token chunks per core
O_TILES = OSH // P  # 16 o-tiles of 128 outf
KB_GROUPS = 4  # k-block phases
KB_SIZE = K_TILES // KB_GROUPS  # 8 k-tiles per phase

# exec time of the slowest core from the last traced run (ns), None if untraced
LAST_EXEC_NS = None
LAST_RESULT = None


def _install_axon_ntff_hook():
    """Best-effort: register the axon NTFF profiling hook that the image's
    antenv package lacks. Returns True if tracing is possible."""
    try:
        from antenv.axon_hooks import get_axon_ntff_profile_hook

        return get_axon_ntff_profile_hook() is not None
    except ImportError:
        pass
    try:
        import sys
        import types

        import antenv
        import trn_agent_boot.trn_boot as tb

        hook = tb._ntff_profile_via_ctypes("/opt/axon/libaxon_pjrt.so")
        if hook is None:
            return False
        mod = types.ModuleType("antenv.axon_hooks")
        mod._hook = hook
        mod.get_axon_ntff_profile_hook = lambda: mod._hook
        mod.set_axon_ntff_profile_hook = lambda h: setattr(mod, "_hook", h)
        sys.modules["antenv.axon_hooks"] = mod
        antenv.axon_hooks = mod

        # avoid the artifact-upload dependency in the trace path
        import concourse.bass_utils as bu

        bu.upload_artifacts = lambda tmpdir: str(tmpdir)
        return True
    except Exception:
        return False


def _build_bass():
    nc = bacc.Bacc(None, target_bir_lowering=False)

    x_d = nc.dram_tensor(
        "xt", [P, K_TILES, TSH], mybir.dt.float32r, kind="ExternalInput"
    )
    # wt[kb, ot, p(k), k8, o] = W[o0 + ot*128 + o, (kb*8 + k8)*128 + p]
    w_d = nc.dram_tensor(
        "wt",
        [KB_GROUPS, O_TILES, P, KB_SIZE, P],
        mybir.dt.float32r,
        kind="ExternalInput",
    )
    b_d = nc.dram_tensor("bias", [P, O_TILES], mybir.dt.float32, kind="ExternalInput")
    o_d = nc.dram_tensor(
        "out", [O_TILES, P, TSH], mybir.dt.float32, kind="ExternalOutput"
    )

    with tile.TileContext(nc) as tc:
        with (
            tc.tile_pool(name="xpool", bufs=2 * KB_SIZE) as xpool,
            tc.tile_pool(name="wpool", bufs=4) as wpool,
            tc.tile_pool(name="apool", bufs=1) as apool,
            tc.tile_pool(name="bpool", bufs=1) as bpool,
            tc.tile_pool(name="psum", bufs=8, space="PSUM") as ppool,
        ):
            bias_sb = bpool.tile([P, O_TILES], mybir.dt.float32)
            nc.scalar.dma_start(bias_sb[:], b_d[:])

            acc_tiles = [
                apool.tile([P, TSH], mybir.dt.float32, tag=f"a{ot}", name="acc")
                for ot in range(O_TILES)
            ]

            for kb in range(KB_GROUPS):
                x_tiles = []
                for k8 in range(KB_SIZE):
                    k = kb * KB_SIZE + k8
                    x_k = xpool.tile([P, TSH], mybir.dt.float32r, tag="x", name="x")
                    nc.scalar.dma_start(x_k[:], x_d[:, k])
                    x_tiles.append(x_k)

                for ot in range(O_TILES):
                    w_sb = wpool.tile(
                        [P, KB_SIZE, P], mybir.dt.float32r, tag="w", name="w"
                    )
                    nc.sync.dma_start(w_sb[:], w_d[kb, ot])
                    psums = [
                        ppool.tile([P, NFREE], mybir.dt.float32, tag="acc", name="ps")
                        for _ in range(T_CHUNKS)
                    ]
                    for k8 in range(KB_SIZE):
                        for tcn in range(T_CHUNKS):
                            nc.tensor.matmul(
                                psums[tcn][:],
                                lhsT=w_sb[:, k8],
                                rhs=x_tiles[k8][:, tcn * NFREE : (tcn + 1) * NFREE],
                                start=(k8 == 0),
                                stop=(k8 == KB_SIZE - 1),
                            )
                    acc = acc_tiles[ot]
                    for tcn in range(T_CHUNKS):
                        sl = slice(tcn * NFREE, (tcn + 1) * NFREE)
                        if kb == 0:
                            nc.vector.tensor_tensor(
                                acc[:, sl],
                                psums[tcn][:],
                                bias_sb[:, ot : ot + 1].to_broadcast([P, NFREE]),
                                mybir.AluOpType.add,
                            )
                        else:
                            nc.vector.tensor_tensor(
                                acc[:, sl],
                                psums[tcn][:],
                                acc[:, sl],
                                mybir.AluOpType.add,
                            )
                    if kb == KB_GROUPS - 1:
                        nc.scalar.dma_start(o_d[ot], acc[:])

    nc.compile()
    return nc


def _dense_weight(weight_data, block_ids):
    """Scatter nonzero 32x32 blocks into dense [OUT, IN] (numpy, host-side)."""
    w = np.zeros((OUT_FEATURES, IN_FEATURES), dtype=np.float32)
    br = block_ids.astype(np.int64) // IN_BLOCKS
    bc = block_ids.astype(np.int64) % IN_BLOCKS
    # view as [OUT_BLOCKS, 32, IN_BLOCKS, 32] and scatter per-block
    w4 = w.reshape(OUT_BLOCKS, BLOCK, IN_BLOCKS, BLOCK)
    w4[br, :, bc, :] = weight_data
    return w


def kernel(x, weight_data, bias, block_ids):
    x = np.ascontiguousarray(np.asarray(x, dtype=np.float32))
    weight_data = np.asarray(weight_data, dtype=np.float32)
    bias = np.asarray(bias, dtype=np.float32)
    block_ids = np.asarray(block_ids)

    w = _dense_weight(weight_data, block_ids)  # [OUT, IN]

    # per-token-shard x^T in device layout [P, K_TILES, TSH]:
    # xt[p, k, t] = x[t0 + t, k*128 + p]
    xts = []
    for ti in range(T_SHARDS):
        xs = x[ti * TSH : (ti + 1) * TSH, :]  # [TSH, IN]
        xt = np.ascontiguousarray(
            xs.T.reshape(K_TILES, P, TSH).transpose(1, 0, 2)
        )  # [P, K_TILES, TSH]
        xts.append(xt)

    # per-outf-shard W in device layout [KB_GROUPS, O_TILES, P(k), KB_SIZE, P(o)]:
    # wt[kb, ot, p, k8, o] = W[o0 + ot*128 + o, (kb*8 + k8)*128 + p]
    wts = []
    biases = []
    for si in range(O_SHARDS):
        ws = w[si * OSH : (si + 1) * OSH, :]  # [OSH, IN]
        # [ot, o, kb, k8, p] -> [kb, ot, p, k8, o]
        wt = ws.reshape(O_TILES, P, KB_GROUPS, KB_SIZE, P).transpose(2, 0, 4, 3, 1)
        wts.append(np.ascontiguousarray(wt))
        bs = bias[si * OSH : (si + 1) * OSH]  # [OSH]
        biases.append(np.ascontiguousarray(bs.reshape(O_TILES, P).T))  # [P, O_TILES]

    in_maps = []
    for c in range(N_CORES):
        ti, si = c // O_SHARDS, c % O_SHARDS
        in_maps.append({"xt": xts[ti], "wt": wts[si], "bias": biases[si]})

    nc = _build_bass()
    trace = bool(int(os.environ.get("BSL_TRACE", "0")))
    if trace:
        trace = _install_axon_ntff_hook()
    kwargs = {}
    if trace:
        tdir = os.environ.get("BSL_TRACE_DIR")
        if tdir:
            os.makedirs(tdir, exist_ok=True)
            kwargs["tmpdir"] = tdir
        kwargs["trace_cores"] = list(range(N_CORES))
    res = run_bass_kernel_spmd(
        nc,
        in_maps,
        core_ids=list(range(N_CORES)),
        trace=trace,
        **kwargs,
    )

    global LAST_EXEC_NS, LAST_RESULT
    LAST_EXEC_NS = res.exec_time_ns
    LAST_RESULT = res

    out = np.empty((N_TOKENS, OUT_FEATURES), dtype=np.float32)
    for c in range(N_CORES):
        ti, si = c // O_SHARDS, c % O_SHARDS
        o = res.results[c]["out"]  # [O_TILES, P(o), TSH(t)]
        out[ti * TSH : (ti + 1) * TSH, si * OSH : (si + 1) * OSH] = o.reshape(
            OSH, TSH
        ).T
    return out


# revision 11
# speedup vs baseline: 1.2658x; 1.0145x over previous
"""BlockSparseLinear on 8 TRN2 NeuronCores.

Computes out = x @ W_dense.T + bias where W_dense is a [4096, 4096] matrix
assembled from 8192 nonzero 32x32 blocks (50% density).

Strategy:
  - Host: scatter the nonzero blocks into dense per-core weight shards, in the
    exact transposed/tiled DRAM layout the device kernel wants.
  - Sharding: 4-way over tokens x 2-way over out-features (8 cores).
    Per core: out_shard[1024 tokens, 2048 outf] = x_shard @ W_half.T + bias.
  - Device: dense matmul in float32r (FP22 reduced-precision fp32, full PE
    rate at moving-dim >= 256), out^T orientation (psum[o-part, token-free],
    stationary = weight tile, moving = x tile).
  - Phases iterate over k-blocks (8 contraction tiles each) sweeping ALL 16
    o-tiles, with per-o-tile fp32 SBUF accumulators. This spreads the x and W
    HBM traffic evenly across the whole kernel (~12MB per ~58us phase) instead
    of concentrating it in the first phase. Bias is folded into the first
    phase's PSUM->SBUF accumulate; x DMAs ride the ACT HWDGE ring, W DMAs the
    SP ring, so neither queues behind the other.

Per-core loop nest:
  for kb in 4 (k-blocks of 8 k-tiles):
    for ot in 16 (o-tiles of 128 outf):
      psum[tc] for tc in 2                  (2 PSUM banks, pipelined over ot)
      for k in kb, tc in 2:
        matmul(psum[tc], lhsT=w[kb,ot,k], rhs=x_k[tc*512:], start/stop at kb edges)
      acc[ot] (+)= psum (+ bias at kb==0)   (DVE)
      if kb == 3: DMA acc[ot] -> out^T      (ACT ring)
"""

import os

import numpy as np

import concourse.mybir as mybir
import concourse.tile as tile
from concourse import bacc
from concourse.bass_utils import run_bass_kernel_spmd

BLOCK = 32
IN_FEATURES = 4096
OUT_FEATURES = 4096
N_TOKENS = 4096
IN_BLOCKS = IN_FEATURES // BLOCK  # 128
OUT_BLOCKS = OUT_FEATURES // BLOCK  # 128

N_CORES = 8
T_SHARDS = 4  # token shards
O_SHARDS = 2  # out-feature shards
TSH = N_TOKENS // T_SHARDS  # 1024 tokens per core
OSH = OUT_FEATURES // O_SHARDS  # 2048 out features per core

P = 128  # partitions
NFREE = 512  # matmul moving free dim (one PSUM bank of fp32)
K_TILES = IN_FEATURES // P  # 32
T_CHUNKS = TSH // NFREE  # 2 moving token chunks per core
O_TILES = OSH // P  # 16 o-tiles of 128 outf
KB_GROUPS = 4  # k-block phases
KB_SIZE = K_TILES // KB_GROUPS  # 8 k-tiles per phase

# exec time of the slowest core from the last traced run (ns), None if untraced
LAST_EXEC_NS = None
LAST_RESULT = None


def _install_axon_ntff_hook():
    """Best-effort: register the axon NTFF profiling hook that the image's
    antenv package lacks. Returns True if tracing is possible."""
    try:
        from antenv.axon_hooks import get_axon_ntff_profile_hook

        return get_axon_ntff_profile_hook() is not None
    except ImportError:
        pass
    try:
        import sys
        import types

        import antenv
        import trn_agent_boot.trn_boot as tb

        hook = tb._ntff_profile_via_ctypes("/opt/axon/libaxon_pjrt.so")
        if hook is None:
            return False
        mod = types.ModuleType("antenv.axon_hooks")
        mod._hook = hook
        mod.get_axon_ntff_profile_hook = lambda: mod._hook
        mod.set_axon_ntff_profile_hook = lambda h: setattr(mod, "_hook", h)
        sys.modules["antenv.axon_hooks"] = mod
        antenv.axon_hooks = mod

        # avoid the artifact-upload dependency in the trace path
        import concourse.bass_utils as bu

        bu.upload_artifacts = lambda tmpdir: str(tmpdir)
        return True
    except Exception:
        return False


def _build_bass():
    nc = bacc.Bacc(None, target_bir_lowering=False)

    x_d = nc.dram_tensor(
        "xt", [P, K_TILES, TSH], mybir.dt.float32r, kind="ExternalInput"
    )
    # wt[kb, ot, h, p(k), k4, o] = W[o0 + ot*128 + o, (kb*8 + h*4 + k4)*128 + p]
    w_d = nc.dram_tensor(
        "wt",
        [KB_GROUPS, O_TILES, 2, P, KB_SIZE // 2, P],
        mybir.dt.float32r,
        kind="ExternalInput",
    )
    b_d = nc.dram_tensor("bias", [P, O_TILES], mybir.dt.float32, kind="ExternalInput")
    o_d = nc.dram_tensor(
        "out", [O_TILES, P, TSH], mybir.dt.float32, kind="ExternalOutput"
    )

    with tile.TileContext(nc) as tc:
        with (
            tc.tile_pool(name="xpool", bufs=4 * KB_SIZE) as xpool,
            tc.tile_pool(name="wpool", bufs=8) as wpool,
            tc.tile_pool(name="apool", bufs=1) as apool,
            tc.tile_pool(name="bpool", bufs=1) as bpool,
            tc.tile_pool(name="psum", bufs=8, space="PSUM") as ppool,
        ):
            bias_sb = bpool.tile([P, O_TILES], mybir.dt.float32)

            acc_tiles = [
                apool.tile([P, TSH], mybir.dt.float32, tag=f"a{ot}", name="acc")
                for ot in range(O_TILES)
            ]

            for kb in range(KB_GROUPS):
                x_tiles = []
                for k8 in range(KB_SIZE):
                    k = kb * KB_SIZE + k8
                    row = []
                    for tcn in range(T_CHUNKS):
                        x_k = xpool.tile([P, NFREE], mybir.dt.float32r, tag="x", name="x")
                        nc.scalar.dma_start(
                            x_k[:], x_d[:, k, tcn * NFREE : (tcn + 1) * NFREE]
                        )
                        row.append(x_k)
                    x_tiles.append(row)
                if kb == 0:
                    nc.scalar.dma_start(bias_sb[:], b_d[:])

                for ot in range(O_TILES):
                    w_half = []
                    for h in range(2):
                        w_sb = wpool.tile(
                            [P, KB_SIZE // 2, P], mybir.dt.float32r, tag="w", name="w"
                        )
                        nc.sync.dma_start(w_sb[:], w_d[kb, ot, h])
                        w_half.append(w_sb)
                    psums = [
                        ppool.tile([P, NFREE], mybir.dt.float32, tag="acc", name="ps")
                        for _ in range(T_CHUNKS)
                    ]
                    for k8 in range(KB_SIZE):
                        for tcn in range(T_CHUNKS):
                            nc.tensor.matmul(
                                psums[tcn][:],
                                lhsT=w_half[k8 // 4][:, k8 % 4],
                                rhs=x_tiles[k8][tcn][:],
                                start=(k8 == 0),
                                stop=(k8 == KB_SIZE - 1),
                            )
                    acc = acc_tiles[ot]
                    for tcn in range(T_CHUNKS):
                        sl = slice(tcn * NFREE, (tcn + 1) * NFREE)
                        if kb == 0:
                            nc.vector.tensor_tensor(
                                acc[:, sl],
                                psums[tcn][:],
                                bias_sb[:, ot : ot + 1].to_broadcast([P, NFREE]),
                                mybir.AluOpType.add,
                            )
                        else:
                            nc.vector.tensor_tensor(
                                acc[:, sl],
                                psums[tcn][:],
                                acc[:, sl],
                                mybir.AluOpType.add,
                            )
                    if kb == KB_GROUPS - 1:
                        nc.scalar.dma_start(o_d[ot], acc[:])

    nc.compile()
    return nc


def _dense_weight(weight_data, block_ids):
    """Scatter nonzero 32x32 blocks into dense [OUT, IN] (numpy, host-side)."""
    w = np.zeros((OUT_FEATURES, IN_FEATURES), dtype=np.float32)
    br = block_ids.astype(np.int64) // IN_BLOCKS
    bc = block_ids.astype(np.int64) % IN_BLOCKS
    # view as [OUT_BLOCKS, 32, IN_BLOCKS, 32] and scatter per-block
    w4 = w.reshape(OUT_BLOCKS, BLOCK, IN_BLOCKS, BLOCK)
    w4[br, :, bc, :] = weight_data
    return w


def kernel(x, weight_data, bias, block_ids):
    x = np.ascontiguousarray(np.asarray(x, dtype=np.float32))
    weight_data = np.asarray(weight_data, dtype=np.float32)
    bias = np.asarray(bias, dtype=np.float32)
    block_ids = np.asarray(block_ids)

    w = _dense_weight(weight_data, block_ids)  # [OUT, IN]

    # per-token-shard x^T in device layout [P, K_TILES, TSH]:
    # xt[p, k, t] = x[t0 + t, k*128 + p]
    xts = []
    for ti in range(T_SHARDS):
        xs = x[ti * TSH : (ti + 1) * TSH, :]  # [TSH, IN]
        xt = np.ascontiguousarray(
            xs.T.reshape(K_TILES, P, TSH).transpose(1, 0, 2)
        )  # [P, K_TILES, TSH]
        xts.append(xt)

    # per-outf-shard W in device layout [KB_GROUPS, O_TILES, P(k), KB_SIZE, P(o)]:
    # wt[kb, ot, p, k8, o] = W[o0 + ot*128 + o, (kb*8 + k8)*128 + p]
    wts = []
    biases = []
    for si in range(O_SHARDS):
        ws = w[si * OSH : (si + 1) * OSH, :]  # [OSH, IN]
        # [ot, o, kb, h, k4, p] -> [kb, ot, h, p, k4, o]
        wt = ws.reshape(O_TILES, P, KB_GROUPS, 2, KB_SIZE // 2, P).transpose(
            2, 0, 3, 5, 4, 1
        )
        wts.append(np.ascontiguousarray(wt))
        bs = bias[si * OSH : (si + 1) * OSH]  # [OSH]
        biases.append(np.ascontiguousarray(bs.reshape(O_TILES, P).T))  # [P, O_TILES]

    in_maps = []
    for c in range(N_CORES):
        ti, si = c // O_SHARDS, c % O_SHARDS
        in_maps.append({"xt": xts[ti], "wt": wts[si], "bias": biases[si]})

    nc = _build_bass()
    trace = bool(int(os.environ.get("BSL_TRACE", "0")))
    if trace:
        trace = _install_axon_ntff_hook()
    kwargs = {}
    if trace:
        tdir = os.environ.get("BSL_TRACE_DIR")
        if tdir:
            os.makedirs(tdir, exist_ok=True)
            kwargs["tmpdir"] = tdir
        kwargs["trace_cores"] = list(range(N_CORES))
    res = run_bass_kernel_spmd(
        nc,
        in_maps,
        core_ids=list(range(N_CORES)),
        trace=trace,
        **kwargs,
    )

    global LAST_EXEC_NS, LAST_RESULT
    LAST_EXEC_NS = res.exec_time_ns
    LAST_RESULT = res

    out = np.empty((N_TOKENS, OUT_FEATURES), dtype=np.float32)
    for c in range(N_CORES):
        ti, si = c // O_SHARDS, c % O_SHARDS
        o = res.results[c]["out"]  # [O_TILES, P(o), TSH(t)]
        out[ti * TSH : (ti + 1) * TSH, si * OSH : (si + 1) * OSH] = o.reshape(
            OSH, TSH
        ).T
    return out


# revision 12
# speedup vs baseline: 1.2708x; 1.0039x over previous
"""BlockSparseLinear on 8 TRN2 NeuronCores.

Computes out = x @ W_dense.T + bias where W_dense is a [4096, 4096] matrix
assembled from 8192 nonzero 32x32 blocks (50% density).

Strategy:
  - Host: scatter the nonzero blocks into dense per-core weight shards, in the
    exact transposed/tiled DRAM layout the device kernel wants.
  - Sharding: 4-way over tokens x 2-way over out-features (8 cores).
    Per core: out_shard[1024 tokens, 2048 outf] = x_shard @ W_half.T + bias.
  - Device: dense matmul in float32r (FP22 reduced-precision fp32, full PE
    rate at moving-dim >= 256), out^T orientation (psum[o-part, token-free],
    stationary = weight tile, moving = x tile).
  - Phases iterate over k-blocks (8 contraction tiles each) sweeping ALL 16
    o-tiles, with per-o-tile fp32 SBUF accumulators. This spreads the x and W
    HBM traffic evenly across the whole kernel (~12MB per ~58us phase) instead
    of concentrating it in the first phase. Bias is folded into the first
    phase's PSUM->SBUF accumulate; x DMAs ride the ACT HWDGE ring, W DMAs the
    SP ring, so neither queues behind the other.

Per-core loop nest:
  for kb in 4 (k-blocks of 8 k-tiles):
    for ot in 16 (o-tiles of 128 outf):
      psum[tc] for tc in 2                  (2 PSUM banks, pipelined over ot)
      for k in kb, tc in 2:
        matmul(psum[tc], lhsT=w[kb,ot,k], rhs=x_k[tc*512:], start/stop at kb edges)
      acc[ot] (+)= psum (+ bias at kb==0)   (DVE)
      if kb == 3: DMA acc[ot] -> out^T      (ACT ring)
"""

import os

import numpy as np

import concourse.mybir as mybir
import concourse.tile as tile
from concourse import bacc
from concourse.bass_utils import run_bass_kernel_spmd

BLOCK = 32
IN_FEATURES = 4096
OUT_FEATURES = 4096
N_TOKENS = 4096
IN_BLOCKS = IN_FEATURES // BLOCK  # 128
OUT_BLOCKS = OUT_FEATURES // BLOCK  # 128

N_CORES = 8
T_SHARDS = 4  # token shards
O_SHARDS = 2  # out-feature shards
TSH = N_TOKENS // T_SHARDS  # 1024 tokens per core
OSH = OUT_FEATURES // O_SHARDS  # 2048 out features per core

P = 128  # partitions
NFREE = 512  # matmul moving free dim (one PSUM bank of fp32)
K_TILES = IN_FEATURES // P  # 32
T_CHUNKS = TSH // NFREE  # 2 moving token chunks per core
O_TILES = OSH // P  # 16 o-tiles of 128 outf
KB_GROUPS = 4  # k-block phases
KB_SIZE = K_TILES // KB_GROUPS  # 8 k-tiles per phase

# exec time of the slowest core from the last traced run (ns), None if untraced
LAST_EXEC_NS = None
LAST_RESULT = None


def _install_axon_ntff_hook():
    """Best-effort: register the axon NTFF profiling hook that the image's
    antenv package lacks. Returns True if tracing is possible."""
    try:
        from antenv.axon_hooks import get_axon_ntff_profile_hook

        return get_axon_ntff_profile_hook() is not None
    except ImportError:
        pass
    try:
        import sys
        import types

        import antenv
        import trn_agent_boot.trn_boot as tb

        hook = tb._ntff_profile_via_ctypes("/opt/axon/libaxon_pjrt.so")
        if hook is None:
            return False
        mod = types.ModuleType("antenv.axon_hooks")
        mod._hook = hook
        mod.get_axon_ntff_profile_hook = lambda: mod._hook
        mod.set_axon_ntff_profile_hook = lambda h: setattr(mod, "_hook", h)
        sys.modules["antenv.axon_hooks"] = mod
        antenv.axon_hooks = mod

        # avoid the artifact-upload dependency in the trace path
        import concourse.bass_utils as bu

        bu.upload_artifacts = lambda tmpdir: str(tmpdir)
        return True
    except Exception:
        return False


def _build_bass():
    nc = bacc.Bacc(None, target_bir_lowering=False)

    x_d = nc.dram_tensor(
        "xt", [P, K_TILES, TSH], mybir.dt.float32r, kind="ExternalInput"
    )
    # wt[kb, ot, h, p(k), k4, o] = W[o0 + ot*128 + o, (kb*8 + h*4 + k4)*128 + p]
    w_d = nc.dram_tensor(
        "wt",
        [KB_GROUPS, O_TILES, 2, P, KB_SIZE // 2, P],
        mybir.dt.float32r,
        kind="ExternalInput",
    )
    b_d = nc.dram_tensor("bias", [P, O_TILES], mybir.dt.float32, kind="ExternalInput")
    o_d = nc.dram_tensor(
        "out", [O_TILES, P, TSH], mybir.dt.float32, kind="ExternalOutput"
    )

    with tile.TileContext(nc) as tc:
        with (
            tc.tile_pool(name="xpool", bufs=4 * KB_SIZE) as xpool,
            tc.tile_pool(name="wpool", bufs=16) as wpool,
            tc.tile_pool(name="apool", bufs=1) as apool,
            tc.tile_pool(name="bpool", bufs=1) as bpool,
            tc.tile_pool(name="psum", bufs=8, space="PSUM") as ppool,
        ):
            bias_sb = bpool.tile([P, O_TILES], mybir.dt.float32)

            acc_tiles = [
                apool.tile([P, TSH], mybir.dt.float32, tag=f"a{ot}", name="acc")
                for ot in range(O_TILES)
            ]

            for kb in range(KB_GROUPS):
                x_tiles = []
                for k8 in range(KB_SIZE):
                    k = kb * KB_SIZE + k8
                    row = []
                    for tcn in range(T_CHUNKS):
                        x_k = xpool.tile([P, NFREE], mybir.dt.float32r, tag="x", name="x")
                        nc.scalar.dma_start(
                            x_k[:], x_d[:, k, tcn * NFREE : (tcn + 1) * NFREE]
                        )
                        row.append(x_k)
                    x_tiles.append(row)
                if kb == 0:
                    nc.scalar.dma_start(bias_sb[:], b_d[:])

                for ot in range(O_TILES):
                    w_half = []
                    for h in range(2):
                        w_sb = wpool.tile(
                            [P, KB_SIZE // 2, P], mybir.dt.float32r, tag="w", name="w"
                        )
                        nc.sync.dma_start(w_sb[:], w_d[kb, ot, h])
                        w_half.append(w_sb)
                    psums = [
                        ppool.tile([P, NFREE], mybir.dt.float32, tag="acc", name="ps")
                        for _ in range(T_CHUNKS)
                    ]
                    for k8 in range(KB_SIZE):
                        for tcn in range(T_CHUNKS):
                            nc.tensor.matmul(
                                psums[tcn][:],
                                lhsT=w_half[k8 // 4][:, k8 % 4],
                                rhs=x_tiles[k8][tcn][:],
                                start=(k8 == 0),
                                stop=(k8 == KB_SIZE - 1),
                            )
                    acc = acc_tiles[ot]
                    for tcn in range(T_CHUNKS):
                        sl = slice(tcn * NFREE, (tcn + 1) * NFREE)
                        if kb == 0:
                            nc.vector.tensor_tensor(
                                acc[:, sl],
                                psums[tcn][:],
                                bias_sb[:, ot : ot + 1].to_broadcast([P, NFREE]),
                                mybir.AluOpType.add,
                            )
                        else:
                            nc.vector.tensor_tensor(
                                acc[:, sl],
                                psums[tcn][:],
                                acc[:, sl],
                                mybir.AluOpType.add,
                            )
                        if kb == KB_GROUPS - 1:
                            nc.scalar.dma_start(o_d[ot, :, sl], acc[:, sl])

    nc.compile()
    return nc


def _dense_weight(weight_data, block_ids):
    """Scatter nonzero 32x32 blocks into dense [OUT, IN] (numpy, host-side)."""
    w = np.zeros((OUT_FEATURES, IN_FEATURES), dtype=np.float32)
    br = block_ids.astype(np.int64) // IN_BLOCKS
    bc = block_ids.astype(np.int64) % IN_BLOCKS
    # view as [OUT_BLOCKS, 32, IN_BLOCKS, 32] and scatter per-block
    w4 = w.reshape(OUT_BLOCKS, BLOCK, IN_BLOCKS, BLOCK)
    w4[br, :, bc, :] = weight_data
    return w


def kernel(x, weight_data, bias, block_ids):
    x = np.ascontiguousarray(np.asarray(x, dtype=np.float32))
    weight_data = np.asarray(weight_data, dtype=np.float32)
    bias = np.asarray(bias, dtype=np.float32)
    block_ids = np.asarray(block_ids)

    w = _dense_weight(weight_data, block_ids)  # [OUT, IN]

    # per-token-shard x^T in device layout [P, K_TILES, TSH]:
    # xt[p, k, t] = x[t0 + t, k*128 + p]
    xts = []
    for ti in range(T_SHARDS):
        xs = x[ti * TSH : (ti + 1) * TSH, :]  # [TSH, IN]
        xt = np.ascontiguousarray(
            xs.T.reshape(K_TILES, P, TSH).transpose(1, 0, 2)
        )  # [P, K_TILES, TSH]
        xts.append(xt)

    # per-outf-shard W in device layout [KB_GROUPS, O_TILES, P(k), KB_SIZE, P(o)]:
    # wt[kb, ot, p, k8, o] = W[o0 + ot*128 + o, (kb*8 + k8)*128 + p]
    wts = []
    biases = []
    for si in range(O_SHARDS):
        ws = w[si * OSH : (si + 1) * OSH, :]  # [OSH, IN]
        # [ot, o, kb, h, k4, p] -> [kb, ot, h, p, k4, o]
        wt = ws.reshape(O_TILES, P, KB_GROUPS, 2, KB_SIZE // 2, P).transpose(
            2, 0, 3, 5, 4, 1
        )
        wts.append(np.ascontiguousarray(wt))
        bs = bias[si * OSH : (si + 1) * OSH]  # [OSH]
        biases.append(np.ascontiguousarray(bs.reshape(O_TILES, P).T))  # [P, O_TILES]

    in_maps = []
    for c in range(N_CORES):
        ti, si = c // O_SHARDS, c % O_SHARDS
        in_maps.append({"xt": xts[ti], "wt": wts[si], "bias": biases[si]})

    nc = _build_bass()
    trace = bool(int(os.environ.get("BSL_TRACE", "0")))
    if trace:
        trace = _install_axon_ntff_hook()
    kwargs = {}
    if trace:
        tdir = os.environ.get("BSL_TRACE_DIR")
        if tdir:
            os.makedirs(tdir, exist_ok=True)
            kwargs["tmpdir"] = tdir
        kwargs["trace_cores"] = list(range(N_CORES))
    res = run_bass_kernel_spmd(
        nc,
        in_maps,
        core_ids=list(range(N_CORES)),
        trace=trace,
        **kwargs,
    )

    global LAST_EXEC_NS, LAST_RESULT
    LAST_EXEC_NS = res.exec_time_ns
    LAST_RESULT = res

    out = np.empty((N_TOKENS, OUT_FEATURES), dtype=np.float32)
    for c in range(N_CORES):
        ti, si = c // O_SHARDS, c % O_SHARDS
        o = res.results[c]["out"]  # [O_TILES, P(o), TSH(t)]
        out[ti * TSH : (ti + 1) * TSH, si * OSH : (si + 1) * OSH] = o.reshape(
            OSH, TSH
        ).T
    return out
